# revision 1
# baseline (speedup 1.0000x reference)
"""Trainium2 Bass kernel: single-head causal attention.

B=4, T=4096, E=512, H=64, fp32 in/out.

Sharding: 2 cores per batch sample. Each core computes partial softmax
(numerator and denominator) for ALL 4096 queries of its sample over HALF
the keys: core 2b takes even 128-key-strips, core 2b+1 odd strips. This
keeps the SPMD program structurally identical on every core (per-chunk
trip counts don't depend on the core id) and perfectly load-balanced.
The host combines partials: out = (num0+num1)/(den0+den1).

Device kernel per core (all matmul operands bf16, fp32 PSUM accumulate):
  - QKV projections from host-pretransposed x^T tiles.  K^T/V^T are
    produced packed ([Wk|Wv] stationary -> PSUM rows 0:64 = K^T chunk,
    rows 64:128 = V^T chunk).
  - V^T -> V (natural [k,h]) via hardware DMA-transpose (bf16).
  - Scores computed in S^T=[k,q] layout (K^T strip stationary, Q^T
    moving) so the softmax key-sum reduces over the PARTITION dim and
    comes free via a ones-column appended to V in the PV matmul.
  - exp on the scalar engine with fused 1/sqrt(H) scale; no max
    subtraction (scores are bounded; fp32 exp cannot overflow here).
  - Causal mask applied multiplicatively after exp using 2 constant
    mask tiles (per-core data) on the last two strips of each chunk.
"""

import functools
import os

import numpy as np
import ml_dtypes

B, T, E, H = 4, 4096, 512, 64
NCORES = 8
NCHUNK = 8  # 512-query chunks per sample
CHUNK = T // NCHUNK  # 512
NSTRIP = 16  # local 128-key strips per core (half of T/128)
VSTRIDE = 80  # per-strip stride in the packed V tile (65 used, 32B-aligned)

bf16 = ml_dtypes.bfloat16


@functools.lru_cache(maxsize=1)
def _build():
    import concourse.mybir as mybir
    from concourse import bacc
    import concourse.tile as tile

    dt_bf = mybir.dt.bfloat16
    dt_f32 = mybir.dt.float32

    nc = bacc.Bacc("TRN2", target_bir_lowering=False, num_devices=NCORES)

    xt = nc.dram_tensor("xt", [4, 128, T], dt_bf, kind="ExternalInput")
    xtk = nc.dram_tensor("xtk", [4, 128, T // 2], dt_bf, kind="ExternalInput")
    wq = nc.dram_tensor("wq", [128, 4 * 64], dt_bf, kind="ExternalInput")
    wkv = nc.dram_tensor("wkv", [128, 4 * 128], dt_bf, kind="ExternalInput")
    bias_q = nc.dram_tensor("bias_q", [64, 1], dt_f32, kind="ExternalInput")
    bias_kv = nc.dram_tensor("bias_kv", [128, 1], dt_f32, kind="ExternalInput")
    masks = nc.dram_tensor("masks", [128, 2 * CHUNK], dt_bf, kind="ExternalInput")
    out_d = nc.dram_tensor("out", [H + 1, T], dt_f32, kind="ExternalOutput")

    with tile.TileContext(nc) as tc:
        with (
            tc.tile_pool(name="const", bufs=1) as cpool,
            tc.tile_pool(name="xt_pool", bufs=6) as xpool,
            tc.tile_pool(name="q_pool", bufs=NCHUNK) as qpool,
            tc.tile_pool(name="kv_pool", bufs=4) as kvpool,
            tc.tile_pool(name="v_pool", bufs=1) as vpool,
            tc.tile_pool(name="p_pool", bufs=2) as ppool,
            tc.tile_pool(name="o_pool", bufs=1) as opool,
            tc.tile_pool(name="ps_q", bufs=1, space="PSUM") as psq_pool,
            tc.tile_pool(name="ps_kv", bufs=1, space="PSUM") as pskv_pool,
            tc.tile_pool(name="ps_s", bufs=1, space="PSUM") as pss_pool,
            tc.tile_pool(name="ps_o", bufs=2, space="PSUM") as pso_pool,
        ):
            # ---- persistent constants ----
            wq_sb = cpool.tile([128, 4 * 64], dt_bf)
            nc.sync.dma_start(wq_sb, wq.ap())
            wkv_sb = cpool.tile([128, 4 * 128], dt_bf)
            nc.sync.dma_start(wkv_sb, wkv.ap())
            bq_sb = cpool.tile([64, 1], dt_f32)
            nc.sync.dma_start(bq_sb, bias_q.ap())
            bkv_sb = cpool.tile([128, 1], dt_f32)
            nc.sync.dma_start(bkv_sb, bias_kv.ap())
            masks_sb = cpool.tile([128, 2 * CHUNK], dt_bf)
            nc.sync.dma_start(masks_sb, masks.ap())

            # packed V (natural [k,h] layout + ones column for the denominator)
            v_nat = vpool.tile([128, NSTRIP * VSTRIDE], dt_bf)
            v3 = v_nat.rearrange("p (s c) -> p s c", c=VSTRIDE)
            nc.vector.memset(v3[:, :, 64:65], 1.0)

            out_sb = opool.tile([H + 1, T], dt_f32)

            # ---- K/V projections over this core's 2048 keys ----
            kv_tiles = []
            for ckv in range(4):
                ps_kv = pskv_pool.tile([128, CHUNK], dt_f32, tag="pskv")
                for es in range(4):
                    xk_t = xpool.tile([128, CHUNK], dt_bf, tag="xt")
                    nc.sync.dma_start(
                        xk_t, xtk.ap()[es, :, ckv * CHUNK : (ckv + 1) * CHUNK]
                    )
                    nc.tensor.matmul(
                        ps_kv,
                        lhsT=wkv_sb[:, es * 128 : (es + 1) * 128],
                        rhs=xk_t,
                        start=(es == 0),
                        stop=(es == 3),
                    )
                kv_sb = kvpool.tile([128, CHUNK], dt_bf, tag="kv")
                nc.vector.tensor_scalar_add(kv_sb, ps_kv, bkv_sb)
                kv_tiles.append(kv_sb)
                # V^T (rows 64:128) -> natural V strips via DMA transpose
                for j in range(4):
                    s = 4 * ckv + j
                    nc.sync.dma_start(
                        out=v_nat[:, s * VSTRIDE : s * VSTRIDE + 64],
                        in_=kv_sb[64:128, j * 128 : (j + 1) * 128],
                        transpose=True,
                    )

            # ---- Q projections over all 4096 queries ----
            q_tiles = []
            for c in range(NCHUNK):
                ps_q = psq_pool.tile([64, CHUNK], dt_f32, tag="psq")
                for es in range(4):
                    x_t = xpool.tile([128, CHUNK], dt_bf, tag="xt")
                    nc.sync.dma_start(
                        x_t, xt.ap()[es, :, c * CHUNK : (c + 1) * CHUNK]
                    )
                    nc.tensor.matmul(
                        ps_q,
                        lhsT=wq_sb[:, es * 64 : (es + 1) * 64],
                        rhs=x_t,
                        start=(es == 0),
                        stop=(es == 3),
                    )
                q_sb = qpool.tile([64, CHUNK], dt_bf, tag="q")
                nc.vector.tensor_scalar_add(q_sb, ps_q, bq_sb)
                q_tiles.append(q_sb)

            # ---- attention: chunk c attends to local strips 0..2c+1 ----
            scale = 1.0 / np.sqrt(H)
            for c in range(NCHUNK):
                ns = 2 * (c + 1)  # local strips for this chunk
                ps_o = pso_pool.tile([H + 1, CHUNK], dt_f32, tag="pso")
                for g0 in range(0, ns, 4):
                    g = min(4, ns - g0)
                    ps_s = pss_pool.tile([128, 4 * CHUNK], dt_f32, tag="pss")
                    for i in range(g):
                        l = g0 + i
                        nc.tensor.matmul(
                            ps_s[:, i * CHUNK : (i + 1) * CHUNK],
                            lhsT=kv_tiles[l // 4][0:64, (l % 4) * 128 : (l % 4 + 1) * 128],
                            rhs=q_tiles[c],
                            start=True,
                            stop=True,
                        )
                    p_sb = ppool.tile([128, 4 * CHUNK], dt_bf, tag="p")
                    nc.scalar.activation(
                        p_sb[:, : g * CHUNK],
                        ps_s[:, : g * CHUNK],
                        mybir.ActivationFunctionType.Exp,
                        scale=scale,
                    )
                    # causal mask on the last two strips (l = 2c, 2c+1)
                    for i in range(g):
                        l = g0 + i
                        if l >= ns - 2:
                            j = l - (ns - 2)
                            nc.vector.tensor_mul(
                                p_sb[:, i * CHUNK : (i + 1) * CHUNK],
                                p_sb[:, i * CHUNK : (i + 1) * CHUNK],
                                masks_sb[:, j * CHUNK : (j + 1) * CHUNK],
                            )
                    for i in range(g):
                        l = g0 + i
                        nc.tensor.matmul(
                            ps_o,
                            lhsT=v_nat[:, l * VSTRIDE : l * VSTRIDE + 65],
                            rhs=p_sb[:, i * CHUNK : (i + 1) * CHUNK],
                            start=(l == 0),
                            stop=(l == ns - 1),
                        )
                nc.vector.tensor_copy(
                    out_sb[:, c * CHUNK : (c + 1) * CHUNK], ps_o
                )

            nc.sync.dma_start(out_d.ap(), out_sb)

    nc.compile()
    return nc


def _make_in_maps(x, Wq, bq, Wk, bk, Wv, bv):
    wq_pack = np.ascontiguousarray(
        Wq.reshape(4, 128, 64).transpose(1, 0, 2).reshape(128, 256)
    ).astype(bf16)
    wkv_pack = np.ascontiguousarray(
        np.concatenate(
            [Wk.reshape(4, 128, 64), Wv.reshape(4, 128, 64)], axis=2
        ).transpose(1, 0, 2).reshape(128, 512)
    ).astype(bf16)
    bias_q = np.ascontiguousarray(bq[:, None]).astype(np.float32)
    bias_kv = np.ascontiguousarray(
        np.concatenate([bk, bv])[:, None]
    ).astype(np.float32)

    kk = np.arange(128)[:, None]
    qq = np.arange(CHUNK)[None, :]

    in_maps = []
    for b in range(B):
        xt_b = np.ascontiguousarray(x[b].T).astype(bf16).reshape(4, 128, T)
        for rho in range(2):
            key_tok = np.concatenate(
                [np.arange(128 * (2 * l + rho), 128 * (2 * l + rho) + 128)
                 for l in range(NSTRIP)]
            )
            xtk_b = np.ascontiguousarray(xt_b[:, :, key_tok])
            m0 = (kk - qq <= -128 * rho).astype(bf16)
            m1 = (kk - qq <= -256 - 128 * rho).astype(bf16)
            masks_np = np.ascontiguousarray(np.concatenate([m0, m1], axis=1))
            in_maps.append(
                {
                    "xt": xt_b,
                    "xtk": xtk_b,
                    "wq": wq_pack,
                    "wkv": wkv_pack,
                    "bias_q": bias_q,
                    "bias_kv": bias_kv,
                    "masks": masks_np,
                }
            )
    return in_maps


def _combine(results):
    out = np.empty((B, T, H), np.float32)
    for b in range(B):
        a0 = results[2 * b]["out"].astype(np.float64)
        a1 = results[2 * b + 1]["out"].astype(np.float64)
        num = a0[:H] + a1[:H]
        den = a0[H] + a1[H]
        out[b] = (num / den).T.astype(np.float32)
    return out


def _run(trace=False, **inputs):
    from concourse import bass_utils

    nc = _build()
    in_maps = _make_in_maps(
        np.asarray(inputs["x"], np.float32),
        np.asarray(inputs["Wq"], np.float32),
        np.asarray(inputs["bq"], np.float32),
        np.asarray(inputs["Wk"], np.float32),
        np.asarray(inputs["bk"], np.float32),
        np.asarray(inputs["Wv"], np.float32),
        np.asarray(inputs["bv"], np.float32),
    )
    res = bass_utils.run_bass_kernel_spmd(
        nc, in_maps, list(range(NCORES)), trace=trace
    )
    return _combine(res.results), res.exec_time_ns


def kernel(**inputs):
    out, _ = _run(trace=False, **inputs)
    return out


# revision 5
# speedup vs baseline: 1.9146x; 1.9146x over previous
"""Trainium2 Bass kernel: single-head causal attention.

B=4, T=4096, E=512, H=64, fp32 in/out.

Sharding: 2 cores per batch sample. Each core computes partial softmax
(numerator and denominator) for ALL 4096 queries of its sample over HALF
the keys: core 2b takes even 128-key-strips, core 2b+1 odd strips. This
keeps the SPMD program structurally identical on every core (per-chunk
trip counts don't depend on the core id) and perfectly load-balanced.
The host combines partials: out = (num0+num1)/(den0+den1).

Device kernel per core (all matmul operands bf16, fp32 PSUM accumulate):
  - x^T resident in SBUF (few large DMAs for full bandwidth).
  - QKV projections; K^T/V^T produced packed ([Wk|Wv] stationary ->
    PSUM rows 0:64 = K^T chunk, rows 64:128 = V^T chunk).
  - V^T -> V (natural [k,h]) via PE transpose + DVE cast.
  - Scores in S^T=[k,q] layout (K^T strip stationary, Q^T moving) so the
    softmax key-sum reduces over the PARTITION dim and comes free via a
    ones-column appended to V in the PV matmul.
  - exp on the scalar engine with fused 1/sqrt(H) scale; no max
    subtraction (scores bounded; fp32 exp cannot overflow here).
  - Causal mask applied multiplicatively after exp using 2 constant
    per-core mask tiles on the last two strips of each chunk.
"""

import functools

import numpy as np
import ml_dtypes

B, T, E, H = 4, 4096, 512, 64
NCORES = 8
NCHUNK = 8  # 512-query chunks per sample
CHUNK = T // NCHUNK  # 512
NSTRIP = 16  # local 128-key strips per core (half of T/128)
VSTRIDE = 80  # per-strip stride in the packed V tile

bf16 = ml_dtypes.bfloat16


@functools.lru_cache(maxsize=1)
def _build():
    import concourse.mybir as mybir
    from concourse import bacc
    from concourse.masks import make_identity
    import concourse.tile as tile

    dt_bf = mybir.dt.bfloat16
    dt_f32 = mybir.dt.float32

    nc = bacc.Bacc("TRN2", target_bir_lowering=False, num_devices=NCORES)

    # host layouts: xt [2 halves, 4 e-strips, 128, 2048 tokens]
    xt = nc.dram_tensor("xt", [2, 4, 128, T // 2], dt_bf, kind="ExternalInput")
    # keys: [2 halves, 4 e-strips, 128, 1024 key-tokens]
    xtk = nc.dram_tensor("xtk", [2, 4, 128, T // 4], dt_bf, kind="ExternalInput")
    wq = nc.dram_tensor("wq", [128, 4 * 64], dt_bf, kind="ExternalInput")
    wkv = nc.dram_tensor("wkv", [128, 4 * 128], dt_bf, kind="ExternalInput")
    bias_q = nc.dram_tensor("bias_q", [64, 1], dt_f32, kind="ExternalInput")
    bias_kv = nc.dram_tensor("bias_kv", [128, 1], dt_f32, kind="ExternalInput")
    masks = nc.dram_tensor("masks", [128, 2 * CHUNK], dt_bf, kind="ExternalInput")
    out_d = nc.dram_tensor("out", [H + 1, T], dt_f32, kind="ExternalOutput")

    with tile.TileContext(nc) as tc:
        with (
            tc.tile_pool(name="const", bufs=1) as cpool,
            tc.tile_pool(name="xt_pool", bufs=1) as xpool,
            tc.tile_pool(name="q_pool", bufs=NCHUNK) as qpool,
            tc.tile_pool(name="kv_pool", bufs=4) as kvpool,
            tc.tile_pool(name="v_pool", bufs=1) as vpool,
            tc.tile_pool(name="p_pool", bufs=3) as ppool,
            tc.tile_pool(name="o_pool", bufs=1) as opool,
            tc.tile_pool(name="ps_proj", bufs=2, space="PSUM") as pspr_pool,
            tc.tile_pool(name="ps_s", bufs=2, space="PSUM") as pss_pool,
            tc.tile_pool(name="ps_o", bufs=2, space="PSUM") as pso_pool,
        ):
            # ---- persistent constants ----
            wkv_sb = cpool.tile([128, 4 * 128], dt_bf)
            nc.sync.dma_start(wkv_sb, wkv.ap())
            wq_sb = cpool.tile([128, 4 * 64], dt_bf)
            nc.sync.dma_start(wq_sb, wq.ap())
            bq_sb = cpool.tile([64, 1], dt_f32)
            nc.sync.dma_start(bq_sb, bias_q.ap())
            bkv_sb = cpool.tile([128, 1], dt_f32)
            nc.sync.dma_start(bkv_sb, bias_kv.ap())
            masks_sb = cpool.tile([128, 2 * CHUNK], dt_bf)
            nc.sync.dma_start(masks_sb, masks.ap())
            ident = cpool.tile([128, 128], dt_bf)
            make_identity(nc, ident)

            # resident x^T: [128, es, tokens]
            xtk_sb = xpool.tile([128, 4, T // 2], dt_bf)
            for h in range(2):
                nc.sync.dma_start(
                    xtk_sb[:, :, h * (T // 4) : (h + 1) * (T // 4)],
                    xtk.ap()[h].rearrange("a p t -> p a t"),
                )
            xt_sb = xpool.tile([128, 4, T], dt_bf)
            for h in range(2):
                nc.sync.dma_start(
                    xt_sb[:, :, h * (T // 2) : (h + 1) * (T // 2)],
                    xt.ap()[h].rearrange("a p t -> p a t"),
                )

            # packed V (natural [k,h] layout + ones column for denominator)
            v_nat = vpool.tile([128, NSTRIP * VSTRIDE], dt_bf)
            v3 = v_nat.rearrange("p (s c) -> p s c", c=VSTRIDE)
            nc.vector.memset(v3[:, :, 64:65], 1.0)

            out_sb = opool.tile([H + 1, T], dt_f32)

            # ---- K/V projections over this core's 2048 keys ----
            kv_tiles = []
            for ckv in range(4):
                ps_kv = pspr_pool.tile([128, CHUNK], dt_f32, tag="proj")
                for es in range(4):
                    nc.tensor.matmul(
                        ps_kv,
                        lhsT=wkv_sb[:, es * 128 : (es + 1) * 128],
                        rhs=xtk_sb[:, es, ckv * CHUNK : (ckv + 1) * CHUNK],
                        start=(es == 0),
                        stop=(es == 3),
                    )
                kv_sb = kvpool.tile([128, CHUNK], dt_bf, tag="kv")
                nc.vector.tensor_scalar_add(kv_sb, ps_kv, bkv_sb)
                kv_tiles.append(kv_sb)
                # V^T (rows 64:128) -> natural V strips via PE transpose
                for j in range(4):
                    s = 4 * ckv + j
                    ps_tr = pspr_pool.tile([128, 128], dt_bf, tag="proj")
                    nc.tensor.transpose(
                        ps_tr, kv_sb[:, j * 128 : (j + 1) * 128], ident
                    )
                    nc.vector.tensor_copy(
                        v_nat[:, s * VSTRIDE : s * VSTRIDE + 64], ps_tr[:, 64:128]
                    )

            # ---- Q projections over all 4096 queries ----
            q_tiles = []
            for c in range(NCHUNK):
                ps_q = pspr_pool.tile([64, CHUNK], dt_f32, tag="proj")
                for es in range(4):
                    nc.tensor.matmul(
                        ps_q,
                        lhsT=wq_sb[:, es * 64 : (es + 1) * 64],
                        rhs=xt_sb[:, es, c * CHUNK : (c + 1) * CHUNK],
                        start=(es == 0),
                        stop=(es == 3),
                    )
                q_sb = qpool.tile([64, CHUNK], dt_bf, tag="q")
                nc.vector.tensor_scalar_add(q_sb, ps_q, bq_sb)
                q_tiles.append(q_sb)

            # ---- attention: chunk c attends to local strips 0..2c+1 ----
            scale = 1.0 / float(np.sqrt(H))
            for c in range(NCHUNK):
                ns = 2 * (c + 1)  # local strips for this chunk
                ps_o = pso_pool.tile([H + 1, CHUNK], dt_f32, tag="pso")
                for g0 in range(0, ns, 2):
                    g = min(2, ns - g0)
                    ps_s = pss_pool.tile([128, 2 * CHUNK], dt_f32, tag="pss")
                    for i in range(g):
                        l = g0 + i
                        nc.tensor.matmul(
                            ps_s[:, i * CHUNK : (i + 1) * CHUNK],
                            lhsT=kv_tiles[l // 4][
                                0:64, (l % 4) * 128 : (l % 4 + 1) * 128
                            ],
                            rhs=q_tiles[c],
                            start=True,
                            stop=True,
                        )
                    p_sb = ppool.tile([128, 2 * CHUNK], dt_bf, tag="p")
                    nc.scalar.activation(
                        p_sb[:, : g * CHUNK],
                        ps_s[:, : g * CHUNK],
                        mybir.ActivationFunctionType.Exp,
                        scale=scale,
                    )
                    # causal mask on the last two strips (l = 2c, 2c+1)
                    for i in range(g):
                        l = g0 + i
                        if l >= ns - 2:
                            j = l - (ns - 2)
                            nc.vector.tensor_mul(
                                p_sb[:, i * CHUNK : (i + 1) * CHUNK],
                                p_sb[:, i * CHUNK : (i + 1) * CHUNK],
                                masks_sb[:, j * CHUNK : (j + 1) * CHUNK],
                            )
                    for i in range(g):
                        l = g0 + i
                        nc.tensor.matmul(
                            ps_o,
                            lhsT=v_nat[:, l * VSTRIDE : l * VSTRIDE + 65],
                            rhs=p_sb[:, i * CHUNK : (i + 1) * CHUNK],
                            start=(l == 0),
                            stop=(l == ns - 1),
                        )
                nc.vector.tensor_copy(out_sb[:, c * CHUNK : (c + 1) * CHUNK], ps_o)

            nc.sync.dma_start(out_d.ap(), out_sb)

    nc.compile()
    return nc


def _make_in_maps(x, Wq, bq, Wk, bk, Wv, bv):
    wq_pack = np.ascontiguousarray(
        Wq.reshape(4, 128, 64).transpose(1, 0, 2).reshape(128, 256)
    ).astype(bf16)
    wkv_pack = np.ascontiguousarray(
        np.concatenate([Wk.reshape(4, 128, 64), Wv.reshape(4, 128, 64)], axis=2)
        .transpose(1, 0, 2)
        .reshape(128, 512)
    ).astype(bf16)
    bias_q = np.ascontiguousarray(bq[:, None]).astype(np.float32)
    bias_kv = np.ascontiguousarray(np.concatenate([bk, bv])[:, None]).astype(
        np.float32
    )

    kk = np.arange(128)[:, None]
    qq = np.arange(CHUNK)[None, :]

    in_maps = []
    for b in range(B):
        xt_b = np.ascontiguousarray(x[b].T).astype(bf16).reshape(4, 128, T)
        xt_in = np.ascontiguousarray(
            xt_b.reshape(4, 128, 2, T // 2).transpose(2, 0, 1, 3)
        )
        for rho in range(2):
            key_tok = np.concatenate(
                [
                    np.arange(128 * (2 * l + rho), 128 * (2 * l + rho) + 128)
                    for l in range(NSTRIP)
                ]
            )
            xtk_b = xt_b[:, :, key_tok]
            xtk_in = np.ascontiguousarray(
                xtk_b.reshape(4, 128, 2, T // 4).transpose(2, 0, 1, 3)
            )
            m0 = (kk - qq <= -128 * rho).astype(bf16)
            m1 = (kk - qq <= -256 - 128 * rho).astype(bf16)
            masks_np = np.ascontiguousarray(np.concatenate([m0, m1], axis=1))
            in_maps.append(
                {
                    "xt": xt_in,
                    "xtk": xtk_in,
                    "wq": wq_pack,
                    "wkv": wkv_pack,
                    "bias_q": bias_q,
                    "bias_kv": bias_kv,
                    "masks": masks_np,
                }
            )
    return in_maps


def _combine(results):
    out = np.empty((B, T, H), np.float32)
    for b in range(B):
        a0 = results[2 * b]["out"].astype(np.float64)
        a1 = results[2 * b + 1]["out"].astype(np.float64)
        num = a0[:H] + a1[:H]
        den = a0[H] + a1[H]
        out[b] = (num / den).T.astype(np.float32)
    return out


def _run(trace=False, **inputs):
    from concourse import bass_utils

    nc = _build()
    in_maps = _make_in_maps(
        np.asarray(inputs["x"], np.float32),
        np.asarray(inputs["Wq"], np.float32),
        np.asarray(inputs["bq"], np.float32),
        np.asarray(inputs["Wk"], np.float32),
        np.asarray(inputs["bk"], np.float32),
        np.asarray(inputs["Wv"], np.float32),
        np.asarray(inputs["bv"], np.float32),
    )
    res = bass_utils.run_bass_kernel_spmd(
        nc, in_maps, list(range(NCORES)), trace=trace
    )
    return _combine(res.results), res.exec_time_ns


def kernel(**inputs):
    out, _ = _run(trace=False, **inputs)
    return out


# revision 9
# speedup vs baseline: 2.1453x; 1.1205x over previous
"""Trainium2 Bass kernel: single-head causal attention.

B=4, T=4096, E=512, H=64, fp32 in/out.

Sharding: 2 cores per batch sample. Each core computes partial softmax
(numerator and denominator) for ALL 4096 queries of its sample over HALF
the keys: core 2b takes even 128-key-strips, core 2b+1 odd strips. This
keeps the SPMD program structurally identical on every core (per-chunk
trip counts don't depend on the core id) and perfectly load-balanced.
The host combines partials: out = (num0+num1)/(den0+den1).

Device kernel per core (all matmul operands bf16, fp32 PSUM accumulate):
  - x^T resident in SBUF (few large DMAs for full bandwidth).
  - QKV projections; K^T/V^T produced packed ([Wk|Wv] stationary ->
    PSUM rows 0:64 = K^T chunk, rows 64:128 = V^T chunk).
  - V^T -> V (natural [k,h]) via PE transpose + DVE cast.
  - Scores in S^T=[k,q] layout (K^T strip stationary, Q^T moving) so the
    softmax key-sum reduces over the PARTITION dim and comes free via a
    ones-column appended to V in the PV matmul.
  - exp on the scalar engine with fused 1/sqrt(H) scale; no max
    subtraction (scores bounded; fp32 exp cannot overflow here).
  - Causal mask applied multiplicatively after exp using 2 constant
    per-core mask tiles on the last two strips of each chunk.
"""

import functools

import numpy as np
import ml_dtypes

B, T, E, H = 4, 4096, 512, 64
NCORES = 8
NCHUNK = 8  # 512-query chunks per sample
CHUNK = T // NCHUNK  # 512
NSTRIP = 16  # local 128-key strips per core (half of T/128)
VSTRIDE = 80  # per-strip stride in the packed V tile

bf16 = ml_dtypes.bfloat16


@functools.lru_cache(maxsize=1)
def _build():
    import concourse.mybir as mybir
    from concourse import bacc
    from concourse.masks import make_identity
    import concourse.tile as tile

    dt_bf = mybir.dt.bfloat16
    dt_f32 = mybir.dt.float32

    nc = bacc.Bacc("TRN2", target_bir_lowering=False, num_devices=NCORES)

    # host layouts: xt [4 quarters, 4 e-strips, 128, 1024 tokens]
    xt = nc.dram_tensor("xt", [4, 4, 128, T // 4], dt_bf, kind="ExternalInput")
    # keys: [2 halves, 4 e-strips, 128, 1024 key-tokens]
    xtk = nc.dram_tensor("xtk", [2, 4, 128, T // 4], dt_bf, kind="ExternalInput")
    wq = nc.dram_tensor("wq", [128, 4 * 64], dt_bf, kind="ExternalInput")
    wkv = nc.dram_tensor("wkv", [128, 4 * 128], dt_bf, kind="ExternalInput")
    bias_q = nc.dram_tensor("bias_q", [64, 1], dt_f32, kind="ExternalInput")
    bias_kv = nc.dram_tensor("bias_kv", [128, 1], dt_f32, kind="ExternalInput")
    masks = nc.dram_tensor("masks", [128, 2 * CHUNK], dt_bf, kind="ExternalInput")
    out_d = nc.dram_tensor("out", [H + 1, T], dt_f32, kind="ExternalOutput")

    with tile.TileContext(nc) as tc:
        with (
            tc.tile_pool(name="const", bufs=1) as cpool,
            tc.tile_pool(name="xt_pool", bufs=1) as xpool,
            tc.tile_pool(name="q_pool", bufs=NCHUNK) as qpool,
            tc.tile_pool(name="kv_pool", bufs=4) as kvpool,
            tc.tile_pool(name="v_pool", bufs=1) as vpool,
            tc.tile_pool(name="p_pool", bufs=3) as ppool,
            tc.tile_pool(name="o_pool", bufs=2) as opool,
            tc.tile_pool(name="ps_proj", bufs=2, space="PSUM") as pspr_pool,
            tc.tile_pool(name="ps_s", bufs=2, space="PSUM") as pss_pool,
            tc.tile_pool(name="ps_o", bufs=2, space="PSUM") as pso_pool,
        ):
            # ---- persistent constants ----
            wkv_sb = cpool.tile([128, 4 * 128], dt_bf)
            nc.sync.dma_start(wkv_sb, wkv.ap())
            wq_sb = cpool.tile([128, 4 * 64], dt_bf)
            nc.sync.dma_start(wq_sb, wq.ap())
            bq_sb = cpool.tile([64, 1], dt_f32)
            nc.sync.dma_start(bq_sb, bias_q.ap())
            bkv_sb = cpool.tile([128, 1], dt_f32)
            nc.sync.dma_start(bkv_sb, bias_kv.ap())
            masks_sb = cpool.tile([128, 2 * CHUNK], dt_bf)
            nc.sync.dma_start(masks_sb, masks.ap())
            ident = cpool.tile([128, 128], dt_bf)
            make_identity(nc, ident)

            # resident x^T: [128, es, tokens] (DMAs emitted inside the chunk loop)
            xtk_sb = xpool.tile([128, 4, T // 2], dt_bf)
            xt_sb = xpool.tile([128, 4, T], dt_bf)

            # packed V (natural [k,h] layout + ones column for denominator)
            v_nat = vpool.tile([128, NSTRIP * VSTRIDE], dt_bf)
            v3 = v_nat.rearrange("p (s c) -> p s c", c=VSTRIDE)
            nc.vector.memset(v3[:, :, 64:65], 1.0)

            scale = 1.0 / float(np.sqrt(H))
            kv_tiles = []
            q_tiles = []
            for c in range(NCHUNK):
                # ---- input DMAs, just in time ----
                if c % 4 == 0:
                    h = c // 4
                    nc.sync.dma_start(
                        xtk_sb[:, :, h * (T // 4) : (h + 1) * (T // 4)],
                        xtk.ap()[h].rearrange("a p t -> p a t"),
                    )
                if c % 2 == 0:
                    qd = c // 2
                    nc.sync.dma_start(
                        xt_sb[:, :, qd * (T // 4) : (qd + 1) * (T // 4)],
                        xt.ap()[qd].rearrange("a p t -> p a t"),
                    )

                # ---- K/V projection for kv chunk c//2 (on even c) ----
                if c % 2 == 0:
                    ckv = c // 2
                    ps_kv = pspr_pool.tile([128, CHUNK], dt_f32, tag="proj")
                    for es in range(4):
                        nc.tensor.matmul(
                            ps_kv,
                            lhsT=wkv_sb[:, es * 128 : (es + 1) * 128],
                            rhs=xtk_sb[:, es, ckv * CHUNK : (ckv + 1) * CHUNK],
                            start=(es == 0),
                            stop=(es == 3),
                        )
                    kv_sb = kvpool.tile([128, CHUNK], dt_bf, tag="kv")
                    nc.vector.tensor_scalar_add(kv_sb, ps_kv, bkv_sb)
                    kv_tiles.append(kv_sb)
                    # V^T (rows 64:128) -> natural V strips via PE transpose
                    for j in range(4):
                        s = 4 * ckv + j
                        ps_tr = pspr_pool.tile([128, 128], dt_bf, tag="proj")
                        nc.tensor.transpose(
                            ps_tr, kv_sb[:, j * 128 : (j + 1) * 128], ident
                        )
                        nc.vector.tensor_copy(
                            v_nat[:, s * VSTRIDE : s * VSTRIDE + 64],
                            ps_tr[:, 64:128],
                        )

                # ---- Q projection for chunk c ----
                ps_q = pspr_pool.tile([64, CHUNK], dt_f32, tag="proj")
                for es in range(4):
                    nc.tensor.matmul(
                        ps_q,
                        lhsT=wq_sb[:, es * 64 : (es + 1) * 64],
                        rhs=xt_sb[:, es, c * CHUNK : (c + 1) * CHUNK],
                        start=(es == 0),
                        stop=(es == 3),
                    )
                q_sb = qpool.tile([64, CHUNK], dt_bf, tag="q")
                nc.vector.tensor_scalar_add(q_sb, ps_q, bq_sb)
                q_tiles.append(q_sb)

                # ---- attention: chunk c attends to local strips 0..2c+1 ----
                ns = 2 * (c + 1)
                ps_o = pso_pool.tile([H + 1, CHUNK], dt_f32, tag="pso")
                for g0 in range(0, ns, 2):
                    g = min(2, ns - g0)
                    ps_s = pss_pool.tile([128, 2 * CHUNK], dt_f32, tag="pss")
                    for i in range(g):
                        l = g0 + i
                        nc.tensor.matmul(
                            ps_s[:, i * CHUNK : (i + 1) * CHUNK],
                            lhsT=kv_tiles[l // 4][
                                0:64, (l % 4) * 128 : (l % 4 + 1) * 128
                            ],
                            rhs=q_tiles[c],
                            start=True,
                            stop=True,
                        )
                    p_sb = ppool.tile([128, 2 * CHUNK], dt_bf, tag="p")
                    nc.scalar.activation(
                        p_sb[:, : g * CHUNK],
                        ps_s[:, : g * CHUNK],
                        mybir.ActivationFunctionType.Exp,
                        scale=scale,
                    )
                    # causal mask on the last two strips (l = 2c, 2c+1)
                    for i in range(g):
                        l = g0 + i
                        if l >= ns - 2:
                            j = l - (ns - 2)
                            nc.vector.tensor_mul(
                                p_sb[:, i * CHUNK : (i + 1) * CHUNK],
                                p_sb[:, i * CHUNK : (i + 1) * CHUNK],
                                masks_sb[:, j * CHUNK : (j + 1) * CHUNK],
                            )
                    for i in range(g):
                        l = g0 + i
                        nc.tensor.matmul(
                            ps_o,
                            lhsT=v_nat[:, l * VSTRIDE : l * VSTRIDE + 65],
                            rhs=p_sb[:, i * CHUNK : (i + 1) * CHUNK],
                            start=(l == 0),
                            stop=(l == ns - 1),
                        )
                o_sb = opool.tile([H + 1, CHUNK], dt_f32, tag="o")
                nc.vector.tensor_copy(o_sb, ps_o)
                nc.sync.dma_start(
                    out_d.ap()[:, c * CHUNK : (c + 1) * CHUNK], o_sb
                )

    nc.compile()
    return nc


def _make_in_maps(x, Wq, bq, Wk, bk, Wv, bv):
    wq_pack = np.ascontiguousarray(
        Wq.reshape(4, 128, 64).transpose(1, 0, 2).reshape(128, 256)
    ).astype(bf16)
    wkv_pack = np.ascontiguousarray(
        np.concatenate([Wk.reshape(4, 128, 64), Wv.reshape(4, 128, 64)], axis=2)
        .transpose(1, 0, 2)
        .reshape(128, 512)
    ).astype(bf16)
    bias_q = np.ascontiguousarray(bq[:, None]).astype(np.float32)
    bias_kv = np.ascontiguousarray(np.concatenate([bk, bv])[:, None]).astype(
        np.float32
    )

    kk = np.arange(128)[:, None]
    qq = np.arange(CHUNK)[None, :]

    in_maps = []
    for b in range(B):
        xt_b = np.ascontiguousarray(x[b].T).astype(bf16).reshape(4, 128, T)
        xt_in = np.ascontiguousarray(
            xt_b.reshape(4, 128, 4, T // 4).transpose(2, 0, 1, 3)
        )
        for rho in range(2):
            key_tok = np.concatenate(
                [
                    np.arange(128 * (2 * l + rho), 128 * (2 * l + rho) + 128)
                    for l in range(NSTRIP)
                ]
            )
            xtk_b = xt_b[:, :, key_tok]
            xtk_in = np.ascontiguousarray(
                xtk_b.reshape(4, 128, 2, T // 4).transpose(2, 0, 1, 3)
            )
            m0 = (kk - qq <= -128 * rho).astype(bf16)
            m1 = (kk - qq <= -256 - 128 * rho).astype(bf16)
            masks_np = np.ascontiguousarray(np.concatenate([m0, m1], axis=1))
            in_maps.append(
                {
                    "xt": xt_in,
                    "xtk": xtk_in,
                    "wq": wq_pack,
                    "wkv": wkv_pack,
                    "bias_q": bias_q,
                    "bias_kv": bias_kv,
                    "masks": masks_np,
                }
            )
    return in_maps


def _combine(results):
    out = np.empty((B, T, H), np.float32)
    for b in range(B):
        a0 = results[2 * b]["out"].astype(np.float64)
        a1 = results[2 * b + 1]["out"].astype(np.float64)
        num = a0[:H] + a1[:H]
        den = a0[H] + a1[H]
        out[b] = (num / den).T.astype(np.float32)
    return out


def _run(trace=False, **inputs):
    from concourse import bass_utils

    nc = _build()
    in_maps = _make_in_maps(
        np.asarray(inputs["x"], np.float32),
        np.asarray(inputs["Wq"], np.float32),
        np.asarray(inputs["bq"], np.float32),
        np.asarray(inputs["Wk"], np.float32),
        np.asarray(inputs["bk"], np.float32),
        np.asarray(inputs["Wv"], np.float32),
        np.asarray(inputs["bv"], np.float32),
    )
    res = bass_utils.run_bass_kernel_spmd(
        nc, in_maps, list(range(NCORES)), trace=trace
    )
    return _combine(res.results), res.exec_time_ns


def kernel(**inputs):
    out, _ = _run(trace=False, **inputs)
    return out


# revision 11
# speedup vs baseline: 2.2101x; 1.0302x over previous
"""Trainium2 Bass kernel: single-head causal attention.

B=4, T=4096, E=512, H=64, fp32 in/out.

Sharding: 2 cores per batch sample. Each core computes partial softmax
(numerator and denominator) for ALL 4096 queries of its sample over HALF
the keys: core 2b takes even 128-key-strips, core 2b+1 odd strips. This
keeps the SPMD program structurally identical on every core (per-chunk
trip counts don't depend on the core id) and perfectly load-balanced.
The host combines partials: out = (num0+num1)/(den0+den1).

Device kernel per core (all matmul operands bf16, fp32 PSUM accumulate):
  - x^T resident in SBUF (few large DMAs for full bandwidth).
  - QKV projections; K^T/V^T produced packed ([Wk|Wv] stationary ->
    PSUM rows 0:64 = K^T chunk, rows 64:128 = V^T chunk).
  - V^T -> V (natural [k,h]) via PE transpose + DVE cast.
  - Scores in S^T=[k,q] layout (K^T strip stationary, Q^T moving) so the
    softmax key-sum reduces over the PARTITION dim and comes free via a
    ones-column appended to V in the PV matmul.
  - exp on the scalar engine with fused 1/sqrt(H) scale; no max
    subtraction (scores bounded; fp32 exp cannot overflow here).
  - Causal mask applied multiplicatively after exp using 2 constant
    per-core mask tiles on the last two strips of each chunk.
"""

import functools

import numpy as np
import ml_dtypes

B, T, E, H = 4, 4096, 512, 64
NCORES = 8
NCHUNK = 8  # 512-query chunks per sample
CHUNK = T // NCHUNK  # 512
NSTRIP = 16  # local 128-key strips per core (half of T/128)
VSTRIDE = 80  # per-strip stride in the packed V tile

bf16 = ml_dtypes.bfloat16


@functools.lru_cache(maxsize=1)
def _build():
    import concourse.mybir as mybir
    from concourse import bacc
    from concourse.masks import make_identity
    import concourse.tile as tile

    dt_bf = mybir.dt.bfloat16
    dt_f32 = mybir.dt.float32

    nc = bacc.Bacc("TRN2", target_bir_lowering=False, num_devices=NCORES)

    # host layouts: xt [4 quarters, 4 e-strips, 128, 1024 tokens]
    xt = nc.dram_tensor("xt", [4, 4, 128, T // 4], dt_bf, kind="ExternalInput")
    # keys: [2 halves, 4 e-strips, 128, 1024 key-tokens]
    xtk = nc.dram_tensor("xtk", [2, 4, 128, T // 4], dt_bf, kind="ExternalInput")
    wq = nc.dram_tensor("wq", [128, 4 * 64], dt_bf, kind="ExternalInput")
    wkv = nc.dram_tensor("wkv", [128, 4 * 128], dt_bf, kind="ExternalInput")
    bias_q = nc.dram_tensor("bias_q", [64, 1], dt_f32, kind="ExternalInput")
    bias_kv = nc.dram_tensor("bias_kv", [128, 1], dt_f32, kind="ExternalInput")
    masks = nc.dram_tensor("masks", [128, 2 * CHUNK], dt_bf, kind="ExternalInput")
    out_d = nc.dram_tensor("out", [H + 1, T], dt_f32, kind="ExternalOutput")

    with tile.TileContext(nc) as tc:
        with (
            tc.tile_pool(name="const", bufs=1) as cpool,
            tc.tile_pool(name="xt_pool", bufs=1) as xpool,
            tc.tile_pool(name="q_pool", bufs=NCHUNK) as qpool,
            tc.tile_pool(name="kv_pool", bufs=4) as kvpool,
            tc.tile_pool(name="v_pool", bufs=1) as vpool,
            tc.tile_pool(name="p_pool", bufs=3) as ppool,
            tc.tile_pool(name="o_pool", bufs=2) as opool,
            tc.tile_pool(name="ps_proj", bufs=2, space="PSUM") as pspr_pool,
            tc.tile_pool(name="ps_s", bufs=2, space="PSUM") as pss_pool,
            tc.tile_pool(name="ps_o", bufs=2, space="PSUM") as pso_pool,
        ):
            # ---- input DMAs, upfront, in dependency-priority order ----
            xtk_sb = xpool.tile([128, 4, T // 2], dt_bf)
            xt_sb = xpool.tile([128, 4, T], dt_bf)
            wkv_sb = cpool.tile([128, 4 * 128], dt_bf)
            nc.sync.dma_start(wkv_sb, wkv.ap())
            nc.sync.dma_start(
                xtk_sb[:, :, 0 : T // 4], xtk.ap()[0].rearrange("a p t -> p a t")
            )
            wq_sb = cpool.tile([128, 4 * 64], dt_bf)
            nc.sync.dma_start(wq_sb, wq.ap())
            nc.sync.dma_start(
                xt_sb[:, :, 0 : T // 4], xt.ap()[0].rearrange("a p t -> p a t")
            )
            bq_sb = cpool.tile([64, 1], dt_f32)
            nc.sync.dma_start(bq_sb, bias_q.ap())
            bkv_sb = cpool.tile([128, 1], dt_f32)
            nc.sync.dma_start(bkv_sb, bias_kv.ap())
            masks_sb = cpool.tile([128, 2 * CHUNK], dt_bf)
            nc.sync.dma_start(masks_sb, masks.ap())
            nc.sync.dma_start(
                xt_sb[:, :, T // 4 : T // 2], xt.ap()[1].rearrange("a p t -> p a t")
            )
            nc.sync.dma_start(
                xtk_sb[:, :, T // 4 : T // 2], xtk.ap()[1].rearrange("a p t -> p a t")
            )
            nc.sync.dma_start(
                xt_sb[:, :, T // 2 : 3 * T // 4],
                xt.ap()[2].rearrange("a p t -> p a t"),
            )
            nc.sync.dma_start(
                xt_sb[:, :, 3 * T // 4 : T], xt.ap()[3].rearrange("a p t -> p a t")
            )
            ident = cpool.tile([128, 128], dt_bf)
            make_identity(nc, ident)

            # packed V (natural [k,h] layout + ones column for denominator)
            v_nat = vpool.tile([128, NSTRIP * VSTRIDE], dt_bf)
            v3 = v_nat.rearrange("p (s c) -> p s c", c=VSTRIDE)
            nc.vector.memset(v3[:, :, 64:65], 1.0)

            scale = 1.0 / float(np.sqrt(H))
            kv_tiles = []
            q_tiles = []
            for c in range(NCHUNK):
                # ---- K/V projection for kv chunk c//2 (on even c) ----
                if c % 2 == 0:
                    ckv = c // 2
                    ps_kv = pspr_pool.tile([128, CHUNK], dt_f32, tag="proj")
                    for es in range(4):
                        nc.tensor.matmul(
                            ps_kv,
                            lhsT=wkv_sb[:, es * 128 : (es + 1) * 128],
                            rhs=xtk_sb[:, es, ckv * CHUNK : (ckv + 1) * CHUNK],
                            start=(es == 0),
                            stop=(es == 3),
                        )
                    kv_sb = kvpool.tile([128, CHUNK], dt_bf, tag="kv")
                    nc.vector.tensor_scalar_add(kv_sb, ps_kv, bkv_sb)
                    kv_tiles.append(kv_sb)
                    # V^T (rows 64:128) -> natural V strips via PE transpose
                    for j in range(4):
                        s = 4 * ckv + j
                        ps_tr = pspr_pool.tile([128, 128], dt_bf, tag="proj")
                        nc.tensor.transpose(
                            ps_tr, kv_sb[:, j * 128 : (j + 1) * 128], ident
                        )
                        nc.vector.tensor_copy(
                            v_nat[:, s * VSTRIDE : s * VSTRIDE + 64],
                            ps_tr[:, 64:128],
                        )

                # ---- Q projection for chunk c ----
                ps_q = pspr_pool.tile([64, CHUNK], dt_f32, tag="proj")
                for es in range(4):
                    nc.tensor.matmul(
                        ps_q,
                        lhsT=wq_sb[:, es * 64 : (es + 1) * 64],
                        rhs=xt_sb[:, es, c * CHUNK : (c + 1) * CHUNK],
                        start=(es == 0),
                        stop=(es == 3),
                    )
                q_sb = qpool.tile([64, CHUNK], dt_bf, tag="q")
                nc.vector.tensor_scalar_add(q_sb, ps_q, bq_sb)
                q_tiles.append(q_sb)

                # ---- attention: chunk c attends to local strips 0..2c+1 ----
                ns = 2 * (c + 1)
                ps_o = pso_pool.tile([H + 1, CHUNK], dt_f32, tag="pso")
                for g0 in range(0, ns, 2):
                    g = min(2, ns - g0)
                    ps_s = pss_pool.tile([128, 2 * CHUNK], dt_f32, tag="pss")
                    for i in range(g):
                        l = g0 + i
                        nc.tensor.matmul(
                            ps_s[:, i * CHUNK : (i + 1) * CHUNK],
                            lhsT=kv_tiles[l // 4][
                                0:64, (l % 4) * 128 : (l % 4 + 1) * 128
                            ],
                            rhs=q_tiles[c],
                            start=True,
                            stop=True,
                        )
                    p_sb = ppool.tile([128, 2 * CHUNK], dt_bf, tag="p")
                    nc.scalar.activation(
                        p_sb[:, : g * CHUNK],
                        ps_s[:, : g * CHUNK],
                        mybir.ActivationFunctionType.Exp,
                        scale=scale,
                    )
                    # causal mask on the last two strips (l = 2c, 2c+1)
                    for i in range(g):
                        l = g0 + i
                        if l >= ns - 2:
                            j = l - (ns - 2)
                            nc.vector.tensor_mul(
                                p_sb[:, i * CHUNK : (i + 1) * CHUNK],
                                p_sb[:, i * CHUNK : (i + 1) * CHUNK],
                                masks_sb[:, j * CHUNK : (j + 1) * CHUNK],
                            )
                    for i in range(g):
                        l = g0 + i
                        nc.tensor.matmul(
                            ps_o,
                            lhsT=v_nat[:, l * VSTRIDE : l * VSTRIDE + 65],
                            rhs=p_sb[:, i * CHUNK : (i + 1) * CHUNK],
                            start=(l == 0),
                            stop=(l == ns - 1),
                        )
                o_sb = opool.tile([H + 1, CHUNK], dt_f32, tag="o")
                nc.vector.tensor_copy(o_sb, ps_o)
                nc.sync.dma_start(
                    out_d.ap()[:, c * CHUNK : (c + 1) * CHUNK], o_sb
                )

    nc.compile()
    return nc


def _make_in_maps(x, Wq, bq, Wk, bk, Wv, bv):
    wq_pack = np.ascontiguousarray(
        Wq.reshape(4, 128, 64).transpose(1, 0, 2).reshape(128, 256)
    ).astype(bf16)
    wkv_pack = np.ascontiguousarray(
        np.concatenate([Wk.reshape(4, 128, 64), Wv.reshape(4, 128, 64)], axis=2)
        .transpose(1, 0, 2)
        .reshape(128, 512)
    ).astype(bf16)
    bias_q = np.ascontiguousarray(bq[:, None]).astype(np.float32)
    bias_kv = np.ascontiguousarray(np.concatenate([bk, bv])[:, None]).astype(
        np.float32
    )

    kk = np.arange(128)[:, None]
    qq = np.arange(CHUNK)[None, :]

    in_maps = []
    for b in range(B):
        xt_b = np.ascontiguousarray(x[b].T).astype(bf16).reshape(4, 128, T)
        xt_in = np.ascontiguousarray(
            xt_b.reshape(4, 128, 4, T // 4).transpose(2, 0, 1, 3)
        )
        for rho in range(2):
            key_tok = np.concatenate(
                [
                    np.arange(128 * (2 * l + rho), 128 * (2 * l + rho) + 128)
                    for l in range(NSTRIP)
                ]
            )
            xtk_b = xt_b[:, :, key_tok]
            xtk_in = np.ascontiguousarray(
                xtk_b.reshape(4, 128, 2, T // 4).transpose(2, 0, 1, 3)
            )
            m0 = (kk - qq <= -128 * rho).astype(bf16)
            m1 = (kk - qq <= -256 - 128 * rho).astype(bf16)
            masks_np = np.ascontiguousarray(np.concatenate([m0, m1], axis=1))
            in_maps.append(
                {
                    "xt": xt_in,
                    "xtk": xtk_in,
                    "wq": wq_pack,
                    "wkv": wkv_pack,
                    "bias_q": bias_q,
                    "bias_kv": bias_kv,
                    "masks": masks_np,
                }
            )
    return in_maps


def _combine(results):
    out = np.empty((B, T, H), np.float32)
    for b in range(B):
        a0 = results[2 * b]["out"].astype(np.float64)
        a1 = results[2 * b + 1]["out"].astype(np.float64)
        num = a0[:H] + a1[:H]
        den = a0[H] + a1[H]
        out[b] = (num / den).T.astype(np.float32)
    return out


def _run(trace=False, **inputs):
    from concourse import bass_utils

    nc = _build()
    in_maps = _make_in_maps(
        np.asarray(inputs["x"], np.float32),
        np.asarray(inputs["Wq"], np.float32),
        np.asarray(inputs["bq"], np.float32),
        np.asarray(inputs["Wk"], np.float32),
        np.asarray(inputs["bk"], np.float32),
        np.asarray(inputs["Wv"], np.float32),
        np.asarray(inputs["bv"], np.float32),
    )
    res = bass_utils.run_bass_kernel_spmd(
        nc, in_maps, list(range(NCORES)), trace=trace
    )
    return _combine(res.results), res.exec_time_ns


def kernel(**inputs):
    out, _ = _run(trace=False, **inputs)
    return out
